# revision 6
# baseline (speedup 1.0000x reference)
"""Trainium2 Bass kernel v2.2 for the 2-layer GAT node-classification head.

Mask-neighborhood collapse (only mask_idx's row is read downstream), head-i
on core-i sharding, one AllGather of the folded [v1p, 4] partial.

Perf structure:
  - TWO input DMAs per core: a small bf16 const pack (f32 consts ride along
    via bitcast) and the 1.18MB bf16 W1 head block; edge tile packed to the
    ng*gmax real slots.
  - Xg row blocks from PE transposes of Xg^T chunks (no second x DMA).
  - attention entirely off the critical path: leaky-relu/exp as single Act
    ops, softmax normalization folded into the alpha-selection as a second
    per-partition scalar (group 1/sum broadcast to slots via a PE matmul),
    pad masking folded into an extra m01t row.
  - agg^T = W1^T @ (Xg^T A): 36 tiny lhsT matmuls, then a 4-op whole-tile
    elu (b1 folded per-r only when nonzero), then the folded layer-2 rhs.
  - post-collective: core-sum with ccons/oxm folded in as extra reduction
    rows, Act lrelu, exp, and a divide instead of reciprocal+mul.
"""

import numpy as np

import concourse.bass as bass
import concourse.mybir as mybir
import concourse.tile as tile
from concourse import bacc
from concourse.bass_utils import run_bass_kernel_spmd
from concourse.masks import make_identity

NCORES = 8
P = 128
C = 768          # input feature dim
H1 = 8           # layer-1 heads
OUT = 768        # per-head feature dim
KC = C // P      # 6 k-chunks of 128 over a 768 contraction
W2F = 4          # folded layer-2 cols: [cls0 cls1 a_src2 a_dst2]
NEG = -1.0e30    # padding logit

f32 = mybir.dt.float32
bf16 = mybir.dt.bfloat16
ALU = mybir.AluOpType
ACT = mybir.ActivationFunctionType


# ---------------------------------------------------------------- host graph
def _preprocess(edge_index, mask_idx, n_nodes):
    """2-hop in-neighborhood of mask_idx packed into ng*gmax edge slots."""
    ei = np.asarray(edge_index).astype(np.int64)
    m = int(np.asarray(mask_idx))
    src_all = np.concatenate([ei[0], np.arange(n_nodes, dtype=np.int64)])
    dst_all = np.concatenate([ei[1], np.arange(n_nodes, dtype=np.int64)])

    s1_pos = np.nonzero(dst_all == m)[0]          # in-edges of m (incl self)
    s1_src = src_all[s1_pos].tolist()
    v1 = list(dict.fromkeys(s1_src))              # unique sources
    v1n = len(v1)
    v1_row = {v: r for r, v in enumerate(v1)}
    s1n = len(s1_src)
    # this build implements only the packed small-neighborhood paths
    assert s1n == v1n, "duplicate layer-2 sources not supported"
    assert v1n >= 2, "need at least two layer-2 sources"
    v1p = v1n

    groups = [src_all[np.nonzero(dst_all == v)[0]].tolist() for v in v1]
    gmax = max(len(g) for g in groups)
    ng = len(groups)
    assert ng == v1p
    s2e = ng * gmax
    assert s2e <= P, f"edge slots {s2e} exceed {P}"
    assert v1p * W2F <= 512

    src_ids = np.zeros(s2e, np.int64)             # padded with node 0
    m01 = np.zeros((s2e, v1p), np.float32)
    padrow = np.full((1, s2e), NEG, np.float32)   # extra m01t row
    for j, srcs in enumerate(groups):
        lo = j * gmax
        src_ids[lo:lo + len(srcs)] = srcs
        m01[lo:lo + len(srcs), j] = 1.0
        padrow[0, lo:lo + len(srcs)] = 0.0

    m01te = np.concatenate([np.ascontiguousarray(m01.T), padrow], axis=0)
    meta = dict(m=m, v1n=v1n, v1p=v1p, s1n=s1n, gmax=gmax, ng=ng, s2e=s2e,
                mrow=v1_row[m])
    host = dict(src_ids=src_ids,
                v1_ids=np.array(v1, np.int64),
                m01=m01, m01te=m01te)
    return meta, host


def _chunkT(w):
    """[768, N] -> [128, KC*N] chunk-major free layout (lhsT/rhs tiles)."""
    k, n = w.shape
    assert k == C
    return np.ascontiguousarray(
        w.reshape(KC, P, n).transpose(1, 0, 2).reshape(P, KC * n))


def _cb_layout(meta):
    """Single bf16 const tensor; f32 pieces ride as bitcast (2 cols / f32)."""
    v1p, s2e = meta["v1p"], meta["s2e"]
    pieces = [
        ("xvt", P, KC * v1p),       # Xv^T chunked, for a_dst
        ("ws1", P, KC),             # Ws1[:, head] chunked (lhsT cols)
        ("wd1", P, KC),             # Wd1[:, head] chunked (rhs col)
        ("m01", s2e, v1p),          # edge-slot -> v one-hot
        ("m01te", v1p + 1, s2e),    # transpose + pad-bias row
        ("w2f", P, KC * W2F),       # folded layer-2 rhs, chunked
        ("xm", P, KC),              # x[m] chunked (lhsT col)
        ("wfb", P, KC * 2),         # fc bottom fold, chunked
        ("xgt", P, KC * s2e),       # Xg^T chunked (a_src rhs + transposes)
        # f32 section (bitcast views): 2 bf16 cols per f32 value
        ("b1c", P, 2 * KC),         # b1 head block, column chunks (f32)
        ("ccons", 1, 2 * W2F * v1p),
        ("bias3", 1, 2 * 2),
    ]
    lay, off = {}, 0
    for name, rows, cols in pieces:
        if off % 2:
            off += 1
        lay[name] = (rows, off, cols)
        off += cols
    return lay, off + off % 2


# ---------------------------------------------------------------- bass build
def _build(meta):
    v1p, gmax, ng = meta["v1p"], meta["gmax"], meta["ng"]
    s2e, mrow, s1n = meta["s2e"], meta["mrow"], meta["s1n"]
    b1z = meta["b1z"]
    ccw = W2F * v1p
    NR = NCORES + 2                 # + ccons row + oxm row
    lay, bw = _cb_layout(meta)

    nc = bacc.Bacc("TRN2", target_bir_lowering=False, debug=False,
                   enable_asserts=True, num_devices=NCORES)

    d_cb = nc.dram_tensor("cb", [P, bw], bf16, kind="ExternalInput")
    d_w1 = nc.dram_tensor("w1", [P, KC * KC * P], bf16, kind="ExternalInput")
    d_res = nc.dram_tensor("res", [1, 2], f32, kind="ExternalOutput")

    with tile.TileContext(nc) as tc:
        with (
            tc.tile_pool(name="const", bufs=1) as cpool,
            tc.tile_pool(name="sbuf", bufs=1) as sb,
            tc.tile_pool(name="ps", bufs=1, space="PSUM") as ps,
            tc.tile_pool(name="pst", bufs=2, space="PSUM") as pst,
            tc.tile_pool(name="dram", bufs=1, space="DRAM") as dr,
        ):
            cb = cpool.tile([P, bw], bf16, tag="cb")
            nc.sync.dma_start(out=cb[:], in_=d_cb[:])
            w1_sb = cpool.tile([P, KC, KC, P], bf16, tag="w1")
            nc.sync.dma_start(out=w1_sb[:], in_=d_w1[:].rearrange(
                "p (r c q) -> p r c q", r=KC, c=KC))

            def cbv(name):
                rows, off, cols = lay[name]
                return cb[0:rows, off:off + cols]

            def cfv(name):
                return cbv(name).bitcast(f32)

            xvt_v = cbv("xvt").rearrange("p (k n) -> p k n", k=KC)
            ws1_v = cbv("ws1")
            wd1_v = cbv("wd1")
            m01_v = cbv("m01")
            m01te_v = cbv("m01te")
            w2f_v = cbv("w2f").rearrange("p (k n) -> p k n", k=KC)
            xm_v = cbv("xm")
            wfb_v = cbv("wfb").rearrange("p (k n) -> p k n", k=KC)
            b1c_v = cfv("b1c")
            xgt_v = cbv("xgt").rearrange("p (k n) -> p k n", k=KC)

            # ---- attention for this core's head (all off critical path) ----
            adv_sb = sb.tile([v1p + 1, 1], bf16, tag="adv_sb")
            nc.vector.memset(adv_sb[:], 1.0)
            adv_ps = ps.tile([v1p, 1], f32, tag="small", name="adv")
            for c in range(KC):
                nc.tensor.matmul(out=adv_ps[:], lhsT=xvt_v[:, c, :],
                                 rhs=wd1_v[:, c:c + 1],
                                 start=(c == 0), stop=(c == KC - 1))
            nc.scalar.activation(out=adv_sb[0:v1p, :], in_=adv_ps[:],
                                 func=ACT.Copy)
            lg_ps = ps.tile([1, s2e], f32, tag="lgbt", name="lg")
            for c in range(KC):
                nc.tensor.matmul(out=lg_ps[:], lhsT=ws1_v[:, c:c + 1],
                                 rhs=xgt_v[:, c, :],
                                 start=(c == 0), stop=False)
            nc.tensor.matmul(out=lg_ps[:], lhsT=adv_sb[:], rhs=m01te_v,
                             start=False, stop=True)
            # alpha-hat = exp(leaky_relu(logits)); pad slots -> exp(-inf) = 0
            ll = sb.tile([1, s2e], f32, tag="ll")
            nc.scalar.activation(out=ll[:], in_=lg_ps[:], func=ACT.Prelu,
                                 alpha=0.2)
            lle = sb.tile([1, s2e], f32, tag="lle")
            nc.scalar.activation(out=lle[:], in_=ll[:], func=ACT.Exp)
            # normalized alpha row: group sums, reciprocal, broadcast-mult
            gview = lle[:].rearrange("a (g e) -> a g e", e=gmax)
            sm = sb.tile([1, ng], f32, tag="sm")
            nc.vector.reduce_sum(out=sm[:], in_=gview,
                                 axis=mybir.AxisListType.X)
            rc = sb.tile([1, ng], f32, tag="rc")
            nc.vector.reciprocal(out=rc[:], in_=sm[:])
            an = sb.tile([1, s2e], f32, tag="an")
            nc.vector.tensor_tensor(
                out=an[:].rearrange("a (g e) -> a g e", e=gmax), in0=gview,
                in1=rc[:].rearrange("a (g o) -> a g o", o=1).to_broadcast(
                    [1, ng, gmax]),
                op=ALU.mult)
            onesr = sb.tile([1, P], f32, tag="onesr")
            nc.gpsimd.memset(onesr[:], 1.0)
            anb_ps = ps.tile([P, s2e], f32, tag="anb", name="anb")
            nc.tensor.matmul(out=anb_ps[:], lhsT=onesr[:], rhs=an[:],
                             start=True, stop=True)
            anb_sb = sb.tile([P, s2e], f32, tag="anb_sb")
            nc.scalar.activation(out=anb_sb[:], in_=anb_ps[:], func=ACT.Copy)

            # ---- B^T = Xg^T alpha-scaled + group-summed (pre-W1, on DVE) ----
            xga = sb.tile([P, KC, ng * gmax], f32, tag="xga")
            nc.vector.tensor_tensor(
                out=xga[:],
                in0=xgt_v[:].rearrange("p k n -> p k n"),
                in1=anb_sb[:].rearrange(
                    "p (a x) -> p a x", a=1).to_broadcast(
                    [P, KC, ng * gmax]),
                op=ALU.mult)
            xgs = sb.tile([P, KC * ng], bf16, tag="xgs")
            with nc.allow_low_precision(reason="bf16 matmul rhs; <=5 terms"):
                nc.vector.reduce_sum(
                    out=xgs[:].rearrange("p (kg o) -> p kg o", o=1),
                    in_=xga[:].rearrange("p k (g e) -> p (k g) e", e=gmax),
                    axis=mybir.AxisListType.X)
            xgs_v = xgs[:].rearrange("p (k g) -> p k g", k=KC)
            # ---- agg^T = W1^T @ B^T: 36 tiny lhsT matmuls at W1 arrival ----
            agg_ps = ps.tile([P, KC * v1p], f32, tag="agg", name="agg")
            for r in range(KC):
                for c in range(KC):
                    nc.tensor.matmul(out=agg_ps[:, r * v1p:(r + 1) * v1p],
                                     lhsT=w1_sb[:, r, c, :],
                                     rhs=xgs_v[:, c, :],
                                     start=(c == 0), stop=(c == KC - 1))
            # ---- elu+1 = max(h,0) + min(exp(h),1); -1 folded into ccons.
            # The two halves go straight to bf16 and the h2f contraction
            # accumulates both, so no fused-add sits on the critical path
            # and the relu half's matmuls start before exp finishes. ----
            elu = sb.tile([P, KC, v1p], bf16, tag="elu")
            eview = elu[:].rearrange("p k n -> p (k n)")
            split_elu = b1z
            if b1z:
                eh = sb.tile([P, KC * v1p], f32, tag="eh")
                nc.scalar.activation(out=eh[:], in_=agg_ps[:], func=ACT.Exp)
                mxb = sb.tile([P, KC, v1p], bf16, tag="mxb")
                nc.vector.tensor_scalar_max(
                    out=mxb[:].rearrange("p k n -> p (k n)"),
                    in0=agg_ps[:], scalar1=0.0)
                nc.vector.tensor_scalar_min(out=eview, in0=eh[:],
                                            scalar1=1.0)
            else:
                for r in range(KC):
                    rps = agg_ps[:, r * v1p:(r + 1) * v1p]
                    mn = sb.tile([P, v1p], f32, tag=f"mn{r}", name=f"mn{r}")
                    nc.vector.tensor_scalar(out=mn[:], in0=rps,
                                            scalar1=b1c_v[:, r:r + 1],
                                            scalar2=0.0, op0=ALU.add,
                                            op1=ALU.min)
                    mx = sb.tile([P, v1p], f32, tag=f"mx{r}", name=f"mx{r}")
                    nc.vector.tensor_scalar(out=mx[:], in0=rps,
                                            scalar1=b1c_v[:, r:r + 1],
                                            scalar2=0.0, op0=ALU.add,
                                            op1=ALU.max)
                    nc.scalar.activation(out=mn[:], in_=mn[:], func=ACT.Exp)
                    nc.vector.tensor_add(out=elu[:, r, :], in0=mx[:],
                                         in1=mn[:])
            # ---- folded layer-2 partial ----
            h2f_ps = ps.tile([v1p, W2F], f32, tag="small", name="h2f")
            if split_elu:
                for r in range(KC):
                    nc.tensor.matmul(out=h2f_ps[:], lhsT=mxb[:, r, :],
                                     rhs=w2f_v[:, r, :],
                                     start=(r == 0), stop=False)
                for r in range(KC):
                    nc.tensor.matmul(out=h2f_ps[:], lhsT=elu[:, r, :],
                                     rhs=w2f_v[:, r, :],
                                     start=False, stop=(r == KC - 1))
            else:
                for r in range(KC):
                    nc.tensor.matmul(out=h2f_ps[:], lhsT=elu[:, r, :],
                                     rhs=w2f_v[:, r, :],
                                     start=(r == 0), stop=(r == KC - 1))


            # ---- xm @ wfb + bias3 (independent of the collective) ----
            oxm_ps = ps.tile([1, 2], f32, tag="small", name="oxm")
            for c in range(KC):
                nc.tensor.matmul(out=oxm_ps[:], lhsT=xm_v[:, c:c + 1],
                                 rhs=wfb_v[:, c, :],
                                 start=(c == 0), stop=(c == KC - 1))
            oxm_sb = sb.tile([1, 2], f32, tag="oxm_sb")
            nc.vector.tensor_add(out=oxm_sb[:], in0=oxm_ps[:],
                                 in1=cfv("bias3"))

            # ---- AllGather of the [v1p, 4] partial (flattened row) ----
            cc_in = dr.tile([1, ccw], f32, tag="cc_in", name="cc_in")
            cc_out = dr.tile([1, NCORES * ccw], f32, tag="cc_out",
                             name="cc_out")
            h2f_sb = sb.tile([v1p, W2F], f32, tag="h2f_sb")
            nc.vector.tensor_copy(out=h2f_sb[:], in_=h2f_ps[:])
            nc.sync.dma_start(
                out=cc_in[0:1, :].rearrange("a (v f) -> (a v) f", v=v1p),
                in_=h2f_sb[:])
            nc.gpsimd.collective_compute(
                "AllGather", mybir.AluOpType.bypass,
                replica_groups=[list(range(NCORES))],
                ins=[cc_in.opt()], outs=[cc_out.opt()])
            # gathered rows + ccons row + oxm row (both staged pre-collective)
            ccg = sb.tile([1, NR * ccw], f32, tag="ccg")
            nc.vector.tensor_copy(out=ccg[:, NCORES * ccw:(NCORES + 1) * ccw],
                                  in_=cfv("ccons"))
            oview = ccg[:, (NCORES + 1) * ccw:].rearrange(
                "a (v f) -> a v f", f=W2F)
            nc.vector.memset(oview[:, :, 2:4], 0.0)
            nc.vector.tensor_copy(
                out=oview[:, :, 0:2],
                in_=oxm_sb[:].rearrange("a (o f) -> a o f", o=1).to_broadcast(
                    [1, v1p, 2]))
            nc.sync.dma_start(out=ccg[:, 0:NCORES * ccw], in_=cc_out[0:1, :])

            # ---- post: reduce over rows, L2 softmax, classifier ----
            red = sb.tile([1, ccw], f32, tag="red")
            nc.vector.reduce_sum(
                out=red[:],
                in_=ccg[:].rearrange("a (r x) -> a x r", r=NR),
                axis=mybir.AxisListType.X)
            rview = red[:].rearrange("a (v f) -> a f v", f=W2F)
            # logits2[e] = a_src2[v_e] + a_dst2[m]: the a_dst2 scalar rides
            # as the activation bias, fusing the add into the leaky relu
            l2 = sb.tile([1, v1p], f32, tag="l2")
            nc.scalar.activation(
                out=l2[:].rearrange("a (o v) -> a o v", o=1),
                in_=rview[:, 2:3, :], func=ACT.Prelu, alpha=0.2,
                bias=red[:, mrow * W2F + 3:mrow * W2F + 4])
            # exp; the Act accumulator yields the softmax denominator free
            e2 = sb.tile([1, v1p], f32, tag="e2")
            s2 = sb.tile([1, 1], f32, tag="s2")
            nc.scalar.activation(out=e2[:, 0:s1n], in_=l2[:, 0:s1n],
                                 func=ACT.Exp, accum_out=s2[:])
            # unnormalized weighted sum of the two cls columns over edges
            prod = sb.tile([1, 2, s1n], f32, tag="prod")
            wb = e2[:, 0:s1n].rearrange(
                "a (o e) -> a o e", o=1).to_broadcast([1, 2, s1n])
            nc.vector.tensor_tensor(out=prod[:], in0=wb,
                                    in1=rview[:, 0:2, 0:s1n],
                                    op=ALU.mult)
            res0 = sb.tile([1, 2], f32, tag="res0")
            nc.vector.reduce_sum(out=res0[:], in_=prod[:],
                                 axis=mybir.AxisListType.X)
            rc2 = sb.tile([1, 1], f32, tag="rc2")
            nc.vector.reciprocal(out=rc2[:], in_=s2[:])
            nc.vector.tensor_scalar_mul(out=res0[:], in0=res0[:],
                                        scalar1=rc2[:])
            nc.sync.dma_start(out=d_res[:], in_=res0[:])

    nc.compile()
    return nc


_CACHE = {}


def _get_nc(meta):
    key = repr(sorted(meta.items()))
    if key not in _CACHE:
        _CACHE[key] = _build(meta)
    return _CACHE[key]


def make_in_maps(**inputs):
    """Host preprocessing: fold weights, gather x, build per-core inputs."""
    x = np.asarray(inputs["x"], np.float32)
    meta, host = _preprocess(inputs["edge_index"], inputs["mask_idx"],
                             x.shape[0])
    v1p, s2e = meta["v1p"], meta["s2e"]

    W1 = np.asarray(inputs["W1"], np.float32)
    att_s1 = np.asarray(inputs["att_src1"], np.float32)
    att_d1 = np.asarray(inputs["att_dst1"], np.float32)
    b1 = np.asarray(inputs["b1"], np.float32)
    W2 = np.asarray(inputs["W2"], np.float32)
    att_s2 = np.asarray(inputs["att_src2"], np.float32)
    att_d2 = np.asarray(inputs["att_dst2"], np.float32)
    b2 = np.asarray(inputs["b2"], np.float32)
    fc_w = np.asarray(inputs["fc_w"], np.float32)
    fc_b = np.asarray(inputs["fc_b"], np.float32)
    cls_w = np.asarray(inputs["cls_w"], np.float32)
    cls_b = np.asarray(inputs["cls_b"], np.float32)

    meta["b1z"] = bool(np.all(b1 == 0.0))

    Ws1 = np.einsum("chf,hf->ch", W1.reshape(C, H1, OUT), att_s1)  # [C, H1]
    Wd1 = np.einsum("chf,hf->ch", W1.reshape(C, H1, OUT), att_d1)
    Ws2 = W2 @ att_s2[0]                                           # [6144]
    Wd2 = W2 @ att_d2[0]
    wf = fc_w @ cls_w                                              # [1536, 2]
    wf_top, wf_bot = wf[:OUT], wf[OUT:]
    w2fold = W2 @ wf_top                                           # [6144, 2]
    bias3 = (b2 @ wf_top + fc_b @ cls_w + cls_b).reshape(1, 2)

    # folded layer-2 rhs per head block: [cls0 cls1 a_src2 a_dst2]
    w2x = np.concatenate(
        [w2fold, Ws2[:, None], Wd2[:, None]], axis=1)              # [6144, 4]
    # elu(-1) fold: subtract column sums of the FULL w2x after the reduce
    ccons = np.tile(-w2x.sum(axis=0), v1p).reshape(1, W2F * v1p)

    xg = x[host["src_ids"]]                                        # [s2e, 768]
    xgt = _chunkT(np.ascontiguousarray(xg.T))                      # [128,6*s2e]
    xv = x[host["v1_ids"]]                                         # [v1p, 768]
    xvt = _chunkT(np.ascontiguousarray(xv.T))                      # [128,6*v1p]

    lay, bw = _cb_layout(meta)
    nbf = mybir.dt.np(bf16)

    def fill(cst, name, arr):
        rows, off, cols = lay[name]
        assert arr.shape == (rows, cols), (name, arr.shape, (rows, cols))
        cst[0:rows, off:off + cols] = arr.astype(nbf)

    def fill_f32(cst, name, arr):
        rows, off, cols = lay[name]
        assert arr.shape == (rows, cols // 2), (name, arr.shape, (rows, cols))
        raw = np.ascontiguousarray(arr.astype(np.float32)).view(np.uint16)
        cst[0:rows, off:off + cols] = raw.view(nbf)

    in_maps = []
    for i in range(NCORES):
        blk = slice(i * OUT, (i + 1) * OUT)
        # lhsT tiles: [p, r, c, q] = W1[c*128+p, r*128+q]
        w1t = np.ascontiguousarray(
            W1[:, blk].reshape(KC, P, KC, P).transpose(1, 2, 0, 3))
        w1t = w1t.reshape(P, KC * KC * P)
        cbm = np.zeros((P, bw), nbf)
        fill(cbm, "xvt", xvt)
        fill(cbm, "ws1", np.ascontiguousarray(Ws1[:, i].reshape(KC, P).T))
        fill(cbm, "wd1", np.ascontiguousarray(Wd1[:, i].reshape(KC, P).T))
        fill(cbm, "m01", host["m01"])
        fill(cbm, "m01te", host["m01te"])
        fill(cbm, "w2f", _chunkT(w2x[blk]))
        fill(cbm, "xm", np.ascontiguousarray(x[meta["m"]].reshape(KC, P).T))
        fill(cbm, "wfb", _chunkT(np.ascontiguousarray(wf_bot)))
        fill(cbm, "xgt", xgt)
        fill_f32(cbm, "b1c", np.ascontiguousarray(b1[blk].reshape(KC, P).T))
        fill_f32(cbm, "ccons", ccons)
        fill_f32(cbm, "bias3", bias3)
        in_maps.append({"cb": cbm, "w1": w1t.astype(nbf)})
    return meta, in_maps


def kernel(**inputs):
    meta, in_maps = make_in_maps(**inputs)
    nc = _get_nc(meta)
    res = run_bass_kernel_spmd(nc, in_maps, core_ids=list(range(NCORES)))
    return res.results[0]["res"].astype(np.float32)
